# revision 5
# baseline (speedup 1.0000x reference)
"""Trainium2 Bass kernel for nn_EuclideanAttentionBlock (edge gather + MLPs).

Self-contained: kernel(**inputs) -> (filter_w_inv [E,128], filter_w_ev [E,3]).

Design (per core, SPMD over 8 cores, edges sharded):
- ev_features quantized to u16 and kept SBUF-resident as a feature-major
  table [128, 25000, 2] (node pairs along d); gathered with the GPSIMD
  ap_gather ucode (no SWDGE descriptors), producing FEATURE-MAJOR data
  directly -> no PE transposes at all.
- Host sorts edges by (sender&1, receiver&1) so each super-tile uses a
  fixed parity -> gather's pair-select is a free strided view.
- sq = (ev_s - ev_r)^2 on DVE; per-degree segment sums are folded into the
  first-layer weights (Wx = segmask @ W1[32:35] * step^2).
- MLPs as f32r matmuls: h^T = W1a^T @ lengths^T + Wx^T @ sq^T (PSUM accum),
  silu+bias on ACT, out^T = W2^T @ h^T, bias on DVE. Outputs written
  transposed and host-transposed back.
"""
import numpy as np

P = 128
EV = 15
NL = 3
NR = 32
H = 128
F = 128
T = 512          # edges per stream-tile
SUP = 2048       # edges per super-tile (4 streams)
NPAIR = 25000    # node pairs (50000 nodes)
MAX_L = 3

_SEGMASK = np.zeros((EV, NL), np.float32)
_s0 = 0
for _l in range(1, MAX_L + 1):
    _d = 2 * _l + 1
    _SEGMASK[_s0 : _s0 + _d, _l - 1] = 1.0
    _s0 += _d

_NC_CACHE = {}


def _build(e_pad, sup_parity, b2_zero=False):
    """Per-core Bass program. sup_parity: list of (ps, pr) per super."""
    from contextlib import ExitStack
    import concourse.bass as bass
    import concourse.bacc as bacc
    import concourse.tile as tile
    from concourse import mybir

    f32 = mybir.dt.float32
    f32r = mybir.dt.float32r
    u16 = mybir.dt.uint16
    i16 = mybir.dt.int16
    nsup = e_pad // SUP

    nc = bacc.Bacc("TRN2", target_bir_lowering=False, debug=False)
    tab = nc.dram_tensor("tab", [P, NPAIR * 2], u16, kind="ExternalInput")
    idx_all = nc.dram_tensor("idx_all", [P, nsup * 64], i16, kind="ExternalInput")
    len4 = nc.dram_tensor("len4", [nsup, P, T], f32r, kind="ExternalInput")
    w1a4i = nc.dram_tensor("w1a4i", [P, H], f32r, kind="ExternalInput")
    wx4i = nc.dram_tensor("wx4i", [P, H], f32r, kind="ExternalInput")
    w1a4e = nc.dram_tensor("w1a4e", [P, H], f32r, kind="ExternalInput")
    wx4e = nc.dram_tensor("wx4e", [P, H], f32r, kind="ExternalInput")
    w2i = nc.dram_tensor("w2i", [H, F], f32r, kind="ExternalInput")
    w2e = nc.dram_tensor("w2e", [H, NL], f32r, kind="ExternalInput")
    b1i = nc.dram_tensor("b1i", [H, 1], f32, kind="ExternalInput")
    b1e = nc.dram_tensor("b1e", [H, 1], f32, kind="ExternalInput")
    b2i = nc.dram_tensor("b2i", [F, 1], f32, kind="ExternalInput")
    b2e = nc.dram_tensor("b2e", [NL, 1], f32, kind="ExternalInput")
    yinvT = nc.dram_tensor("yinvT", [F, e_pad], f32, kind="ExternalOutput")
    yevT = nc.dram_tensor("yevT", [NL, e_pad], f32, kind="ExternalOutput")

    with tile.TileContext(nc) as tc, ExitStack() as ctx:
        cpool = ctx.enter_context(tc.tile_pool(name="const", bufs=1))
        lpool = ctx.enter_context(tc.tile_pool(name="len", bufs=4))
        gpool = ctx.enter_context(tc.tile_pool(name="g", bufs=4))
        dpool = ctx.enter_context(tc.tile_pool(name="d", bufs=4))
        hpool = ctx.enter_context(tc.tile_pool(name="h", bufs=6))
        ypool = ctx.enter_context(tc.tile_pool(name="y", bufs=6))
        epool = ctx.enter_context(tc.tile_pool(name="yev", bufs=3))
        ps_m = ctx.enter_context(tc.tile_pool(name="ps", bufs=8, space="PSUM"))

        tab_t = cpool.tile([P, NPAIR * 2], u16)
        nc.sync.dma_start(out=tab_t[:], in_=tab[:, :])
        idx_t = cpool.tile([P, nsup * 64], i16)
        nc.sync.dma_start(out=idx_t[:], in_=idx_all[:, :])
        w1ai_t = cpool.tile([P, H], f32r)
        wxi_t = cpool.tile([P, H], f32r)
        w1ae_t = cpool.tile([P, H], f32r)
        wxe_t = cpool.tile([P, H], f32r)
        w2i_t = cpool.tile([H, F], f32r)
        w2e_t = cpool.tile([H, NL], f32r)
        b1i_t = cpool.tile([H, 1], f32)
        b1e_t = cpool.tile([H, 1], f32)
        b2i_t = cpool.tile([F, 1], f32)
        b2e_t = cpool.tile([NL, 1], f32)
        for t_, d_ in ((w1ai_t, w1a4i), (wxi_t, wx4i), (w1ae_t, w1a4e),
                       (wxe_t, wx4e), (w2i_t, w2i), (w2e_t, w2e),
                       (b1i_t, b1i), (b1e_t, b1e), (b2i_t, b2i), (b2e_t, b2e)):
            nc.sync.dma_start(out=t_[:], in_=d_[:, :])

        tab_v = tab_t[:].rearrange("p (a b) -> p a b", b=2)

        for s in range(nsup):
            ps, pr = sup_parity[s]
            len_t = lpool.tile([P, T], f32r, tag="len")
            nc.sync.dma_start(out=len_t[:], in_=len4[s])

            gs_t = gpool.tile([P, T * 2], u16, tag="gs")
            gr_t = gpool.tile([P, T * 2], u16, tag="gr")
            for call, g_t in ((0, gs_t), (1, gr_t)):
                nc.gpsimd.ap_gather(
                    out_ap=g_t[:].rearrange("p (a b) -> p a b", b=2),
                    in_ap=tab_v,
                    idxs_ap=idx_t[:, s * 64 + call * 32 : s * 64 + (call + 1) * 32],
                    channels=P, num_elems=NPAIR, d=2, num_idxs=T,
                )
            gs_v = gs_t[:].rearrange("p (a b) -> p a b", b=2)[:, :, ps]
            gr_v = gr_t[:].rearrange("p (a b) -> p a b", b=2)[:, :, pr]
            df_t = dpool.tile([P, T], f32, tag="df")
            nc.vector.tensor_tensor(out=df_t[:], in0=gs_v, in1=gr_v,
                                    op=mybir.AluOpType.subtract)
            sq_t = dpool.tile([P, T], f32r, tag="sq")
            nc.vector.tensor_tensor(out=sq_t[:], in0=df_t[:], in1=df_t[:],
                                    op=mybir.AluOpType.mult)

            yev_sb = epool.tile([NL, SUP], f32, tag="yev")
            for k in range(4):
                o = 32 * k
                e0 = s * SUP + k * T
                for tag, w1a_t, wx_t, w2_t, b1_t, b2_t, mw in (
                    ("i", w1ai_t, wxi_t, w2i_t, b1i_t, b2i_t, F),
                    ("e", w1ae_t, wxe_t, w2e_t, b1e_t, b2e_t, NL),
                ):
                    h_ps = ps_m.tile([H, T], f32, tag="mm", space="PSUM")
                    nc.tensor.matmul(h_ps[:], lhsT=w1a_t[o : o + NR, :],
                                     rhs=len_t[o : o + NR, :],
                                     start=True, stop=False,
                                     tile_position=(o, 0))
                    nc.tensor.matmul(h_ps[:], lhsT=wx_t[o : o + EV, :],
                                     rhs=sq_t[o : o + EV, :],
                                     start=False, stop=True,
                                     tile_position=(o, 0))
                    h_sb = hpool.tile([H, T], f32r, tag="h" + tag)
                    nc.scalar.activation(out=h_sb[:], in_=h_ps[:],
                                         func=mybir.ActivationFunctionType.Silu,
                                         bias=b1_t[:, :1], scale=1.0)
                    y_ps = ps_m.tile([mw, T], f32, tag="mm", space="PSUM")
                    nc.tensor.matmul(y_ps[:], lhsT=w2_t[:], rhs=h_sb[:],
                                     start=True, stop=True)
                    if tag == "i":
                        y_sb = ypool.tile([F, T], f32, tag="y")
                        if b2_zero:
                            nc.vector.tensor_copy(out=y_sb[:], in_=y_ps[:])
                        else:
                            nc.vector.tensor_scalar_add(out=y_sb[:], in0=y_ps[:],
                                                        scalar1=b2_t[:, :1])
                        nc.sync.dma_start(out=yinvT[:, e0 : e0 + T], in_=y_sb[:])
                    else:
                        if b2_zero:
                            nc.vector.tensor_copy(
                                out=yev_sb[:, k * T : (k + 1) * T], in_=y_ps[:])
                        else:
                            nc.vector.tensor_scalar_add(
                                out=yev_sb[:, k * T : (k + 1) * T], in0=y_ps[:],
                                scalar1=b2_t[:, :1])
            nc.sync.dma_start(out=yevT[:, s * SUP : (s + 1) * SUP], in_=yev_sb[:])
    nc.compile()
    return nc


def kernel(**inputs):
    from concourse.bass_utils import run_bass_kernel_spmd

    ev = np.asarray(inputs["ev_features"], np.float32)
    senders = np.asarray(inputs["senders"]).astype(np.int64)
    receivers = np.asarray(inputs["receivers"]).astype(np.int64)
    lengths = np.asarray(inputs["lengths"], np.float32)
    E = senders.shape[0]
    import os
    ncores = int(os.environ.get("K_NCORES", "8"))
    assert E % ncores == 0
    ec = E // ncores

    # quantize ev to u16 (offset cancels in the diff; step^2 folds into Wx)
    lo = float(ev.min())
    hi = float(ev.max())
    step = (hi - lo) / 65535.0 if hi > lo else 1.0
    evq = np.clip(np.rint((ev - lo) / step), 0, 65535).astype(np.uint16)
    evqT = np.zeros((16, 50000), np.uint16)
    evqT[:EV] = evq.T
    tab128 = np.ascontiguousarray(np.tile(evqT, (8, 1)))  # [128, 50000]

    W1i = np.asarray(inputs["W1_inv"], np.float32)
    W1e = np.asarray(inputs["W1_ev"], np.float32)
    s2 = np.float32(step * step)
    wxi = (_SEGMASK @ W1i[NR:]) * s2
    wxe = (_SEGMASK @ W1e[NR:]) * s2
    w1a4 = np.zeros((P, H), np.float32)
    wx4i = np.zeros((P, H), np.float32)
    wx4e = np.zeros((P, H), np.float32)
    for k in range(4):
        w1a4[32 * k : 32 * k + NR] = W1i[:NR]
        wx4i[32 * k : 32 * k + EV] = wxi
        wx4e[32 * k : 32 * k + EV] = wxe
    w1a4e = np.zeros((P, H), np.float32)
    for k in range(4):
        w1a4e[32 * k : 32 * k + NR] = W1e[:NR]

    wcom = {
        "tab": tab128,
        "w2i": np.ascontiguousarray(np.asarray(inputs["W2_inv"], np.float32)),
        "w2e": np.ascontiguousarray(np.asarray(inputs["W2_ev"], np.float32)),
        "b1i": np.asarray(inputs["b1_inv"], np.float32).reshape(H, 1),
        "b1e": np.asarray(inputs["b1_ev"], np.float32).reshape(H, 1),
        "b2i": np.asarray(inputs["b2_inv"], np.float32).reshape(F, 1),
        "b2e": np.asarray(inputs["b2_ev"], np.float32).reshape(NL, 1),
    }

    # group per core by (s&1, r&1); common padded group size across cores
    per_core = []
    for i in range(ncores):
        sl = slice(i * ec, (i + 1) * ec)
        key = ((senders[sl] & 1) * 2 + (receivers[sl] & 1)).astype(np.int64)
        order = np.argsort(key, kind="stable")
        counts = np.bincount(key, minlength=4)
        per_core.append((order, counts))
    gmax = max(int(c.max()) for _, c in per_core)
    g_sup = -(-gmax // SUP)
    e_pad = 4 * g_sup * SUP
    nsup = e_pad // SUP
    sup_parity = []
    for g in range(4):
        sup_parity += [(g >> 1, g & 1)] * g_sup

    in_maps = []
    inv_pos_all = []
    for i in range(ncores):
        sl = slice(i * ec, (i + 1) * ec)
        order, counts = per_core[i]
        s_c, r_c, l_c = senders[sl], receivers[sl], lengths[sl]
        s_p = np.zeros(e_pad, np.int64)
        r_p = np.zeros(e_pad, np.int64)
        l_p = np.zeros((e_pad, NR), np.float32)
        inv_pos = np.empty(ec, np.int64)
        start = 0
        for g in range(4):
            cnt = int(counts[g])
            idxs = order[start : start + cnt]
            blk = g * g_sup * SUP
            s_p[blk : blk + cnt] = s_c[idxs]
            r_p[blk : blk + cnt] = r_c[idxs]
            l_p[blk : blk + cnt] = l_c[idxs]
            if cnt < g_sup * SUP:
                s_p[blk + cnt : blk + g_sup * SUP] = g >> 1
                r_p[blk + cnt : blk + g_sup * SUP] = g & 1
            inv_pos[idxs] = blk + np.arange(cnt)
            start += cnt
        inv_pos_all.append(inv_pos)

        idx_s = (s_p >> 1).astype(np.int16)
        idx_r = (r_p >> 1).astype(np.int16)

        # idx tile [128, nsup*64]: per super: 32 cols senders, 32 receivers;
        # stream k's idxs wrapped [16, 32] at partitions 32k..32k+15
        def lay(a):
            b = a.reshape(nsup, 4, 32, 16)          # [sup, stream, col, lane]
            t = np.zeros((8, 16, nsup, 32), np.int16)
            t[0::2] = b.transpose(1, 3, 0, 2)        # even 16-blocks
            return t.reshape(P, nsup, 32)
        full = np.stack([lay(idx_s), lay(idx_r)], axis=2)  # [128, nsup, 2, 32]
        idx16 = np.ascontiguousarray(full.reshape(P, nsup * 64))

        l4 = l_p.reshape(nsup, 4, T, NR).transpose(0, 1, 3, 2)  # [sup,k,32,T]
        len4 = np.ascontiguousarray(l4.reshape(nsup, P, T))

        m = {"idx_all": idx16, "len4": len4}
        m.update(wcom)
        in_maps.append(m)

    b2_zero = (not np.any(wcom["b2i"])) and (not np.any(wcom["b2e"]))
    key = (e_pad, tuple(sup_parity), b2_zero)
    if key not in _NC_CACHE:
        _NC_CACHE[key] = _build(e_pad, sup_parity, b2_zero=b2_zero)
    nc = _NC_CACHE[key]

    for m in in_maps:
        m["w1a4i"] = w1a4
        m["wx4i"] = wx4i
        m["w1a4e"] = w1a4e
        m["wx4e"] = wx4e

    res = run_bass_kernel_spmd(nc, in_maps, core_ids=list(range(ncores)),
                               trace=False)
    kernel.last_result = res

    out_inv = np.empty((E, F), np.float32)
    out_ev = np.empty((E, NL), np.float32)
    for i in range(ncores):
        yi = np.asarray(res.results[i]["yinvT"])
        ye = np.asarray(res.results[i]["yevT"])
        pos = inv_pos_all[i]
        out_inv[i * ec : (i + 1) * ec] = yi.T[pos]
        out_ev[i * ec : (i + 1) * ec] = ye.T[pos]
    return out_inv, out_ev


# revision 6
# speedup vs baseline: 1.0118x; 1.0118x over previous
"""Trainium2 Bass kernel for nn_EuclideanAttentionBlock (edge gather + MLPs).

Self-contained: kernel(**inputs) -> (filter_w_inv [E,128], filter_w_ev [E,3]).

Design (per core, SPMD over 8 cores, edges sharded):
- ev_features quantized to u16 and kept SBUF-resident as a feature-major
  table [128, 25000, 2] (node pairs along d); gathered with the GPSIMD
  ap_gather ucode (no SWDGE descriptors), producing FEATURE-MAJOR data
  directly -> no PE transposes at all.
- Host sorts edges by (sender&1, receiver&1) so each super-tile uses a
  fixed parity -> gather's pair-select is a free strided view.
- sq = (ev_s - ev_r)^2 on DVE; per-degree segment sums are folded into the
  first-layer weights (Wx = segmask @ W1[32:35] * step^2).
- MLPs as f32r matmuls: h^T = W1a^T @ lengths^T + Wx^T @ sq^T (PSUM accum),
  silu+bias on ACT, out^T = W2^T @ h^T, bias on DVE. Outputs written
  transposed and host-transposed back.
"""
import numpy as np

P = 128
EV = 15
NL = 3
NR = 32
H = 128
F = 128
T = 512          # edges per stream-tile
SUP = 2048       # edges per super-tile (4 streams)
NPAIR = 25000    # node pairs (50000 nodes)
MAX_L = 3

_SEGMASK = np.zeros((EV, NL), np.float32)
_s0 = 0
for _l in range(1, MAX_L + 1):
    _d = 2 * _l + 1
    _SEGMASK[_s0 : _s0 + _d, _l - 1] = 1.0
    _s0 += _d

_NC_CACHE = {}


def _build(e_pad, sup_parity, b2_zero=False):
    """Per-core Bass program. sup_parity: list of (ps, pr) per super."""
    from contextlib import ExitStack
    import concourse.bass as bass
    import concourse.bacc as bacc
    import concourse.tile as tile
    from concourse import mybir

    f32 = mybir.dt.float32
    f32r = mybir.dt.float32r
    u16 = mybir.dt.uint16
    i16 = mybir.dt.int16
    nsup = e_pad // SUP

    nc = bacc.Bacc("TRN2", target_bir_lowering=False, debug=False)
    tab = nc.dram_tensor("tab", [P, NPAIR * 2], u16, kind="ExternalInput")
    idx_all = nc.dram_tensor("idx_all", [P, nsup * 64], i16, kind="ExternalInput")
    len4 = nc.dram_tensor("len4", [nsup, P, T], f32r, kind="ExternalInput")
    w1a4i = nc.dram_tensor("w1a4i", [P, H], f32r, kind="ExternalInput")
    wx4i = nc.dram_tensor("wx4i", [P, H], f32r, kind="ExternalInput")
    w1a4e = nc.dram_tensor("w1a4e", [P, H], f32r, kind="ExternalInput")
    wx4e = nc.dram_tensor("wx4e", [P, H], f32r, kind="ExternalInput")
    w2i = nc.dram_tensor("w2i", [H, F], f32r, kind="ExternalInput")
    w2e = nc.dram_tensor("w2e", [H, NL], f32r, kind="ExternalInput")
    b1i = nc.dram_tensor("b1i", [H, 1], f32, kind="ExternalInput")
    b1e = nc.dram_tensor("b1e", [H, 1], f32, kind="ExternalInput")
    b2i = nc.dram_tensor("b2i", [F, 1], f32, kind="ExternalInput")
    b2e = nc.dram_tensor("b2e", [NL, 1], f32, kind="ExternalInput")
    yinvT = nc.dram_tensor("yinvT", [F, e_pad], f32, kind="ExternalOutput")
    yevT = nc.dram_tensor("yevT", [NL, e_pad], f32, kind="ExternalOutput")

    with tile.TileContext(nc) as tc, ExitStack() as ctx:
        cpool = ctx.enter_context(tc.tile_pool(name="const", bufs=1))
        lpool = ctx.enter_context(tc.tile_pool(name="len", bufs=4))
        gpool = ctx.enter_context(tc.tile_pool(name="g", bufs=4))
        dpool = ctx.enter_context(tc.tile_pool(name="d", bufs=4))
        hpool = ctx.enter_context(tc.tile_pool(name="h", bufs=6))
        ypool = ctx.enter_context(tc.tile_pool(name="y", bufs=6))
        epool = ctx.enter_context(tc.tile_pool(name="yev", bufs=3))
        ps_m = ctx.enter_context(tc.tile_pool(name="ps", bufs=8, space="PSUM"))

        tab_t = cpool.tile([P, NPAIR * 2], u16)
        nc.sync.dma_start(out=tab_t[:], in_=tab[:, :])
        idx_t = cpool.tile([P, nsup * 64], i16)
        nc.sync.dma_start(out=idx_t[:], in_=idx_all[:, :])
        w1ai_t = cpool.tile([P, H], f32r)
        wxi_t = cpool.tile([P, H], f32r)
        w1ae_t = cpool.tile([P, H], f32r)
        wxe_t = cpool.tile([P, H], f32r)
        w2i_t = cpool.tile([H, F], f32r)
        w2e_t = cpool.tile([H, NL], f32r)
        b1i_t = cpool.tile([H, 1], f32)
        b1e_t = cpool.tile([H, 1], f32)
        b2i_t = cpool.tile([F, 1], f32)
        b2e_t = cpool.tile([NL, 1], f32)
        for t_, d_ in ((w1ai_t, w1a4i), (wxi_t, wx4i), (w1ae_t, w1a4e),
                       (wxe_t, wx4e), (w2i_t, w2i), (w2e_t, w2e),
                       (b1i_t, b1i), (b1e_t, b1e), (b2i_t, b2i), (b2e_t, b2e)):
            nc.sync.dma_start(out=t_[:], in_=d_[:, :])

        tab_v = tab_t[:].rearrange("p (a b) -> p a b", b=2)

        for s in range(nsup):
            ps, pr = sup_parity[s]
            len_t = lpool.tile([P, T], f32r, tag="len")
            nc.sync.dma_start(out=len_t[:], in_=len4[s])

            gs_t = gpool.tile([P, T * 2], u16, tag="gs")
            gr_t = gpool.tile([P, T * 2], u16, tag="gr")
            for call, g_t in ((0, gs_t), (1, gr_t)):
                nc.gpsimd.ap_gather(
                    out_ap=g_t[:].rearrange("p (a b) -> p a b", b=2),
                    in_ap=tab_v,
                    idxs_ap=idx_t[:, s * 64 + call * 32 : s * 64 + (call + 1) * 32],
                    channels=P, num_elems=NPAIR, d=2, num_idxs=T,
                )
            gs_v = gs_t[:].rearrange("p (a b) -> p a b", b=2)[:, :, ps]
            gr_v = gr_t[:].rearrange("p (a b) -> p a b", b=2)[:, :, pr]
            df_t = dpool.tile([P, T], f32, tag="df")
            nc.vector.tensor_tensor(out=df_t[:], in0=gs_v, in1=gr_v,
                                    op=mybir.AluOpType.subtract)
            sq_t = dpool.tile([P, T], f32r, tag="sq")
            nc.vector.tensor_tensor(out=sq_t[:], in0=df_t[:], in1=df_t[:],
                                    op=mybir.AluOpType.mult)

            yev_sb = epool.tile([NL, SUP], f32, tag="yev")
            for k in range(4):
                o = 32 * k
                e0 = s * SUP + k * T
                mlps = (("i", w1ai_t, wxi_t, w2i_t, b1i_t, b2i_t, F),
                        ("e", w1ae_t, wxe_t, w2e_t, b1e_t, b2e_t, NL))
                h_pss = {}
                for tag, w1a_t, wx_t, w2_t, b1_t, b2_t, mw in mlps:
                    h_ps = ps_m.tile([H, T], f32, tag="mm", space="PSUM")
                    nc.tensor.matmul(h_ps[:], lhsT=w1a_t[o : o + NR, :],
                                     rhs=len_t[o : o + NR, :],
                                     start=True, stop=False,
                                     tile_position=(o, 0))
                    nc.tensor.matmul(h_ps[:], lhsT=wx_t[o : o + EV, :],
                                     rhs=sq_t[o : o + EV, :],
                                     start=False, stop=True,
                                     tile_position=(o, 0))
                    h_pss[tag] = h_ps
                h_sbs = {}
                for tag, w1a_t, wx_t, w2_t, b1_t, b2_t, mw in mlps:
                    h_sb = hpool.tile([H, T], f32r, tag="h" + tag)
                    nc.scalar.activation(out=h_sb[:], in_=h_pss[tag][:],
                                         func=mybir.ActivationFunctionType.Silu,
                                         bias=b1_t[:, :1], scale=1.0)
                    h_sbs[tag] = h_sb
                for tag, w1a_t, wx_t, w2_t, b1_t, b2_t, mw in mlps:
                    y_ps = ps_m.tile([mw, T], f32, tag="mm", space="PSUM")
                    nc.tensor.matmul(y_ps[:], lhsT=w2_t[:], rhs=h_sbs[tag][:],
                                     start=True, stop=True)
                    if tag == "i":
                        y_sb = ypool.tile([F, T], f32, tag="y")
                        if b2_zero:
                            nc.vector.tensor_copy(out=y_sb[:], in_=y_ps[:])
                        else:
                            nc.vector.tensor_scalar_add(out=y_sb[:], in0=y_ps[:],
                                                        scalar1=b2_t[:, :1])
                        nc.sync.dma_start(out=yinvT[:, e0 : e0 + T], in_=y_sb[:])
                    else:
                        if b2_zero:
                            nc.vector.tensor_copy(
                                out=yev_sb[:, k * T : (k + 1) * T], in_=y_ps[:])
                        else:
                            nc.vector.tensor_scalar_add(
                                out=yev_sb[:, k * T : (k + 1) * T], in0=y_ps[:],
                                scalar1=b2_t[:, :1])
            nc.sync.dma_start(out=yevT[:, s * SUP : (s + 1) * SUP], in_=yev_sb[:])
    nc.compile()
    return nc


def kernel(**inputs):
    from concourse.bass_utils import run_bass_kernel_spmd

    ev = np.asarray(inputs["ev_features"], np.float32)
    senders = np.asarray(inputs["senders"]).astype(np.int64)
    receivers = np.asarray(inputs["receivers"]).astype(np.int64)
    lengths = np.asarray(inputs["lengths"], np.float32)
    E = senders.shape[0]
    import os
    ncores = int(os.environ.get("K_NCORES", "8"))
    assert E % ncores == 0
    ec = E // ncores

    # quantize ev to u16 (offset cancels in the diff; step^2 folds into Wx)
    lo = float(ev.min())
    hi = float(ev.max())
    step = (hi - lo) / 65535.0 if hi > lo else 1.0
    evq = np.clip(np.rint((ev - lo) / step), 0, 65535).astype(np.uint16)
    evqT = np.zeros((16, 50000), np.uint16)
    evqT[:EV] = evq.T
    tab128 = np.ascontiguousarray(np.tile(evqT, (8, 1)))  # [128, 50000]

    W1i = np.asarray(inputs["W1_inv"], np.float32)
    W1e = np.asarray(inputs["W1_ev"], np.float32)
    s2 = np.float32(step * step)
    wxi = (_SEGMASK @ W1i[NR:]) * s2
    wxe = (_SEGMASK @ W1e[NR:]) * s2
    w1a4 = np.zeros((P, H), np.float32)
    wx4i = np.zeros((P, H), np.float32)
    wx4e = np.zeros((P, H), np.float32)
    for k in range(4):
        w1a4[32 * k : 32 * k + NR] = W1i[:NR]
        wx4i[32 * k : 32 * k + EV] = wxi
        wx4e[32 * k : 32 * k + EV] = wxe
    w1a4e = np.zeros((P, H), np.float32)
    for k in range(4):
        w1a4e[32 * k : 32 * k + NR] = W1e[:NR]

    wcom = {
        "tab": tab128,
        "w2i": np.ascontiguousarray(np.asarray(inputs["W2_inv"], np.float32)),
        "w2e": np.ascontiguousarray(np.asarray(inputs["W2_ev"], np.float32)),
        "b1i": np.asarray(inputs["b1_inv"], np.float32).reshape(H, 1),
        "b1e": np.asarray(inputs["b1_ev"], np.float32).reshape(H, 1),
        "b2i": np.asarray(inputs["b2_inv"], np.float32).reshape(F, 1),
        "b2e": np.asarray(inputs["b2_ev"], np.float32).reshape(NL, 1),
    }

    # group per core by (s&1, r&1); common padded group size across cores
    per_core = []
    for i in range(ncores):
        sl = slice(i * ec, (i + 1) * ec)
        key = ((senders[sl] & 1) * 2 + (receivers[sl] & 1)).astype(np.int64)
        order = np.argsort(key, kind="stable")
        counts = np.bincount(key, minlength=4)
        per_core.append((order, counts))
    gmax = max(int(c.max()) for _, c in per_core)
    g_sup = -(-gmax // SUP)
    e_pad = 4 * g_sup * SUP
    nsup = e_pad // SUP
    sup_parity = []
    for g in range(4):
        sup_parity += [(g >> 1, g & 1)] * g_sup

    in_maps = []
    inv_pos_all = []
    for i in range(ncores):
        sl = slice(i * ec, (i + 1) * ec)
        order, counts = per_core[i]
        s_c, r_c, l_c = senders[sl], receivers[sl], lengths[sl]
        s_p = np.zeros(e_pad, np.int64)
        r_p = np.zeros(e_pad, np.int64)
        l_p = np.zeros((e_pad, NR), np.float32)
        inv_pos = np.empty(ec, np.int64)
        start = 0
        for g in range(4):
            cnt = int(counts[g])
            idxs = order[start : start + cnt]
            blk = g * g_sup * SUP
            s_p[blk : blk + cnt] = s_c[idxs]
            r_p[blk : blk + cnt] = r_c[idxs]
            l_p[blk : blk + cnt] = l_c[idxs]
            if cnt < g_sup * SUP:
                s_p[blk + cnt : blk + g_sup * SUP] = g >> 1
                r_p[blk + cnt : blk + g_sup * SUP] = g & 1
            inv_pos[idxs] = blk + np.arange(cnt)
            start += cnt
        inv_pos_all.append(inv_pos)

        idx_s = (s_p >> 1).astype(np.int16)
        idx_r = (r_p >> 1).astype(np.int16)

        # idx tile [128, nsup*64]: per super: 32 cols senders, 32 receivers;
        # stream k's idxs wrapped [16, 32] at partitions 32k..32k+15
        def lay(a):
            b = a.reshape(nsup, 4, 32, 16)          # [sup, stream, col, lane]
            t = np.zeros((8, 16, nsup, 32), np.int16)
            t[0::2] = b.transpose(1, 3, 0, 2)        # even 16-blocks
            return t.reshape(P, nsup, 32)
        full = np.stack([lay(idx_s), lay(idx_r)], axis=2)  # [128, nsup, 2, 32]
        idx16 = np.ascontiguousarray(full.reshape(P, nsup * 64))

        l4 = l_p.reshape(nsup, 4, T, NR).transpose(0, 1, 3, 2)  # [sup,k,32,T]
        len4 = np.ascontiguousarray(l4.reshape(nsup, P, T))

        m = {"idx_all": idx16, "len4": len4}
        m.update(wcom)
        in_maps.append(m)

    b2_zero = (not np.any(wcom["b2i"])) and (not np.any(wcom["b2e"]))
    key = (e_pad, tuple(sup_parity), b2_zero)
    if key not in _NC_CACHE:
        _NC_CACHE[key] = _build(e_pad, sup_parity, b2_zero=b2_zero)
    nc = _NC_CACHE[key]

    for m in in_maps:
        m["w1a4i"] = w1a4
        m["wx4i"] = wx4i
        m["w1a4e"] = w1a4e
        m["wx4e"] = wx4e

    res = run_bass_kernel_spmd(nc, in_maps, core_ids=list(range(ncores)),
                               trace=False)
    kernel.last_result = res

    out_inv = np.empty((E, F), np.float32)
    out_ev = np.empty((E, NL), np.float32)
    for i in range(ncores):
        yi = np.asarray(res.results[i]["yinvT"])
        ye = np.asarray(res.results[i]["yevT"])
        pos = inv_pos_all[i]
        out_inv[i * ec : (i + 1) * ec] = yi.T[pos]
        out_ev[i * ec : (i + 1) * ec] = ye.T[pos]
    return out_inv, out_ev


# revision 8
# speedup vs baseline: 1.0133x; 1.0014x over previous
"""Trainium2 Bass kernel for nn_EuclideanAttentionBlock (edge gather + MLPs).

Self-contained: kernel(**inputs) -> (filter_w_inv [E,128], filter_w_ev [E,3]).

Design (per core, SPMD over 8 cores, edges sharded):
- ev_features quantized to u16 and kept SBUF-resident as a feature-major
  table [128, 25000, 2] (node pairs along d); gathered with the GPSIMD
  ap_gather ucode (no SWDGE descriptors), producing FEATURE-MAJOR data
  directly -> no PE transposes at all.
- Host sorts edges by (sender&1, receiver&1) so each super-tile uses a
  fixed parity -> gather's pair-select is a free strided view.
- sq = (ev_s - ev_r)^2 on DVE; per-degree segment sums are folded into the
  first-layer weights (Wx = segmask @ W1[32:35] * step^2).
- MLPs as f32r matmuls: h^T = W1a^T @ lengths^T + Wx^T @ sq^T (PSUM accum),
  silu+bias on ACT, out^T = W2^T @ h^T, bias on DVE. Outputs written
  transposed and host-transposed back.
"""
import numpy as np

P = 128
EV = 15
NL = 3
NR = 32
H = 128
F = 128
T = 512          # edges per stream-tile
SUP = 2048       # edges per super-tile (4 streams)
NPAIR = 25000    # node pairs (50000 nodes)
MAX_L = 3

_SEGMASK = np.zeros((EV, NL), np.float32)
_s0 = 0
for _l in range(1, MAX_L + 1):
    _d = 2 * _l + 1
    _SEGMASK[_s0 : _s0 + _d, _l - 1] = 1.0
    _s0 += _d

_NC_CACHE = {}


def _build(e_pad, sup_parity, b2_zero=False):
    """Per-core Bass program. sup_parity: list of (ps, pr) per super."""
    from contextlib import ExitStack
    import concourse.bass as bass
    import concourse.bacc as bacc
    import concourse.tile as tile
    from concourse import mybir

    f32 = mybir.dt.float32
    f32r = mybir.dt.float32r
    u16 = mybir.dt.uint16
    i16 = mybir.dt.int16
    nsup = e_pad // SUP

    nc = bacc.Bacc("TRN2", target_bir_lowering=False, debug=False)
    tab = nc.dram_tensor("tab", [P, NPAIR * 2], u16, kind="ExternalInput")
    idx_all = nc.dram_tensor("idx_all", [P, nsup * 64], i16, kind="ExternalInput")
    len4 = nc.dram_tensor("len4", [nsup, P, T], f32r, kind="ExternalInput")
    w1a4i = nc.dram_tensor("w1a4i", [P, H], f32r, kind="ExternalInput")
    wx4i = nc.dram_tensor("wx4i", [P, H], f32r, kind="ExternalInput")
    w1a4e = nc.dram_tensor("w1a4e", [P, H], f32r, kind="ExternalInput")
    wx4e = nc.dram_tensor("wx4e", [P, H], f32r, kind="ExternalInput")
    w2i = nc.dram_tensor("w2i", [H, F], f32r, kind="ExternalInput")
    w2e = nc.dram_tensor("w2e", [H, NL], f32r, kind="ExternalInput")
    b1i = nc.dram_tensor("b1i", [H, 1], f32, kind="ExternalInput")
    b1e = nc.dram_tensor("b1e", [H, 1], f32, kind="ExternalInput")
    b2i = nc.dram_tensor("b2i", [F, 1], f32, kind="ExternalInput")
    b2e = nc.dram_tensor("b2e", [NL, 1], f32, kind="ExternalInput")
    yinvT = nc.dram_tensor("yinvT", [F, e_pad], f32, kind="ExternalOutput")
    yevT = nc.dram_tensor("yevT", [NL, e_pad], f32, kind="ExternalOutput")

    with tile.TileContext(nc) as tc, ExitStack() as ctx:
        cpool = ctx.enter_context(tc.tile_pool(name="const", bufs=1))
        lpool = ctx.enter_context(tc.tile_pool(name="len", bufs=4))
        gpool = ctx.enter_context(tc.tile_pool(name="g", bufs=3))
        dpool = ctx.enter_context(tc.tile_pool(name="d", bufs=3))
        hpool = ctx.enter_context(tc.tile_pool(name="h", bufs=3))
        ypool = ctx.enter_context(tc.tile_pool(name="y", bufs=3))
        epool = ctx.enter_context(tc.tile_pool(name="yev", bufs=3))
        ps_m = ctx.enter_context(tc.tile_pool(name="ps", bufs=4, space="PSUM"))

        tab_t = cpool.tile([P, NPAIR * 2], u16)
        nc.sync.dma_start(out=tab_t[:], in_=tab[:, :])
        idx_t = cpool.tile([P, nsup * 64], i16)
        nc.sync.dma_start(out=idx_t[:], in_=idx_all[:, :])
        w1ai_t = cpool.tile([P, H], f32r)
        wxi_t = cpool.tile([P, H], f32r)
        w1ae_t = cpool.tile([P, H], f32r)
        wxe_t = cpool.tile([P, H], f32r)
        w2i_t = cpool.tile([H, F], f32r)
        w2e_t = cpool.tile([H, NL], f32r)
        b1i_t = cpool.tile([H, 1], f32)
        b1e_t = cpool.tile([H, 1], f32)
        b2i_t = cpool.tile([F, 1], f32)
        b2e_t = cpool.tile([NL, 1], f32)
        for t_, d_ in ((w1ai_t, w1a4i), (wxi_t, wx4i), (w1ae_t, w1a4e),
                       (wxe_t, wx4e), (w2i_t, w2i), (w2e_t, w2e),
                       (b1i_t, b1i), (b1e_t, b1e), (b2i_t, b2i), (b2e_t, b2e)):
            nc.sync.dma_start(out=t_[:], in_=d_[:, :])

        tab_v = tab_t[:].rearrange("p (a b) -> p a b", b=2)

        for s in range(nsup):
            ps, pr = sup_parity[s]
            len_t = lpool.tile([P, T], f32r, tag="len")
            nc.sync.dma_start(out=len_t[:], in_=len4[s])

            gs_t = gpool.tile([P, T * 2], u16, tag="gs")
            gr_t = gpool.tile([P, T * 2], u16, tag="gr")
            for call, g_t in ((0, gs_t), (1, gr_t)):
                nc.gpsimd.ap_gather(
                    out_ap=g_t[:].rearrange("p (a b) -> p a b", b=2),
                    in_ap=tab_v,
                    idxs_ap=idx_t[:, s * 64 + call * 32 : s * 64 + (call + 1) * 32],
                    channels=P, num_elems=NPAIR, d=2, num_idxs=T,
                )
            gs_v = gs_t[:].rearrange("p (a b) -> p a b", b=2)[:, :, ps]
            gr_v = gr_t[:].rearrange("p (a b) -> p a b", b=2)[:, :, pr]
            df_t = dpool.tile([P, T], f32, tag="df")
            nc.vector.tensor_tensor(out=df_t[:], in0=gs_v, in1=gr_v,
                                    op=mybir.AluOpType.subtract)
            sq_t = dpool.tile([P, T], f32r, tag="sq")
            nc.vector.tensor_tensor(out=sq_t[:], in0=df_t[:], in1=df_t[:],
                                    op=mybir.AluOpType.mult)

            yev_sb = epool.tile([NL, SUP], f32, tag="yev")
            for p2 in range(2):
                ks = (2 * p2, 2 * p2 + 1)
                e0 = s * SUP + ks[0] * T
                mlps = (("i", w1ai_t, wxi_t, w2i_t, b1i_t, b2i_t, F),
                        ("e", w1ae_t, wxe_t, w2e_t, b1e_t, b2e_t, NL))
                h_sbs = {}
                for tag, w1a_t, wx_t, w2_t, b1_t, b2_t, mw in mlps:
                    h_ps = ps_m.tile([H, 2 * T], f32, tag="mm", space="PSUM")
                    for j, k in enumerate(ks):
                        o = 32 * k
                        sl = slice(j * T, (j + 1) * T)
                        nc.tensor.matmul(h_ps[:, sl], lhsT=w1a_t[o : o + NR, :],
                                         rhs=len_t[o : o + NR, :],
                                         start=True, stop=False,
                                         tile_position=(o, 0))
                        nc.tensor.matmul(h_ps[:, sl], lhsT=wx_t[o : o + EV, :],
                                         rhs=sq_t[o : o + EV, :],
                                         start=False, stop=True,
                                         tile_position=(o, 0))
                    h_sb = hpool.tile([H, 2 * T], f32r, tag="h" + tag)
                    nc.scalar.activation(out=h_sb[:], in_=h_ps[:],
                                         func=mybir.ActivationFunctionType.Silu,
                                         bias=b1_t[:, :1], scale=1.0)
                    h_sbs[tag] = h_sb
                for tag, w1a_t, wx_t, w2_t, b1_t, b2_t, mw in mlps:
                    y_ps = ps_m.tile([mw, 2 * T], f32, tag="mm", space="PSUM")
                    for j in range(2):
                        sl = slice(j * T, (j + 1) * T)
                        nc.tensor.matmul(y_ps[:, sl], lhsT=w2_t[:],
                                         rhs=h_sbs[tag][:, sl],
                                         start=True, stop=True)
                    if tag == "i":
                        y_sb = ypool.tile([F, 2 * T], f32, tag="y")
                        if b2_zero:
                            nc.vector.tensor_copy(out=y_sb[:], in_=y_ps[:])
                        else:
                            nc.vector.tensor_scalar_add(out=y_sb[:], in0=y_ps[:],
                                                        scalar1=b2_t[:, :1])
                        nc.sync.dma_start(out=yinvT[:, e0 : e0 + 2 * T],
                                          in_=y_sb[:])
                    else:
                        sl2 = slice(ks[0] * T, (ks[1] + 1) * T)
                        if b2_zero:
                            nc.vector.tensor_copy(out=yev_sb[:, sl2], in_=y_ps[:])
                        else:
                            nc.vector.tensor_scalar_add(out=yev_sb[:, sl2],
                                                        in0=y_ps[:],
                                                        scalar1=b2_t[:, :1])
            nc.sync.dma_start(out=yevT[:, s * SUP : (s + 1) * SUP], in_=yev_sb[:])
    nc.compile()
    return nc


def kernel(**inputs):
    from concourse.bass_utils import run_bass_kernel_spmd

    ev = np.asarray(inputs["ev_features"], np.float32)
    senders = np.asarray(inputs["senders"]).astype(np.int64)
    receivers = np.asarray(inputs["receivers"]).astype(np.int64)
    lengths = np.asarray(inputs["lengths"], np.float32)
    E = senders.shape[0]
    import os
    ncores = int(os.environ.get("K_NCORES", "8"))
    assert E % ncores == 0
    ec = E // ncores

    # quantize ev to u16 (offset cancels in the diff; step^2 folds into Wx)
    lo = float(ev.min())
    hi = float(ev.max())
    step = (hi - lo) / 65535.0 if hi > lo else 1.0
    evq = np.clip(np.rint((ev - lo) / step), 0, 65535).astype(np.uint16)
    evqT = np.zeros((16, 50000), np.uint16)
    evqT[:EV] = evq.T
    tab128 = np.ascontiguousarray(np.tile(evqT, (8, 1)))  # [128, 50000]

    W1i = np.asarray(inputs["W1_inv"], np.float32)
    W1e = np.asarray(inputs["W1_ev"], np.float32)
    s2 = np.float32(step * step)
    wxi = (_SEGMASK @ W1i[NR:]) * s2
    wxe = (_SEGMASK @ W1e[NR:]) * s2
    w1a4 = np.zeros((P, H), np.float32)
    wx4i = np.zeros((P, H), np.float32)
    wx4e = np.zeros((P, H), np.float32)
    for k in range(4):
        w1a4[32 * k : 32 * k + NR] = W1i[:NR]
        wx4i[32 * k : 32 * k + EV] = wxi
        wx4e[32 * k : 32 * k + EV] = wxe
    w1a4e = np.zeros((P, H), np.float32)
    for k in range(4):
        w1a4e[32 * k : 32 * k + NR] = W1e[:NR]

    wcom = {
        "tab": tab128,
        "w2i": np.ascontiguousarray(np.asarray(inputs["W2_inv"], np.float32)),
        "w2e": np.ascontiguousarray(np.asarray(inputs["W2_ev"], np.float32)),
        "b1i": np.asarray(inputs["b1_inv"], np.float32).reshape(H, 1),
        "b1e": np.asarray(inputs["b1_ev"], np.float32).reshape(H, 1),
        "b2i": np.asarray(inputs["b2_inv"], np.float32).reshape(F, 1),
        "b2e": np.asarray(inputs["b2_ev"], np.float32).reshape(NL, 1),
    }

    # group per core by (s&1, r&1); common padded group size across cores
    per_core = []
    for i in range(ncores):
        sl = slice(i * ec, (i + 1) * ec)
        key = ((senders[sl] & 1) * 2 + (receivers[sl] & 1)).astype(np.int64)
        order = np.argsort(key, kind="stable")
        counts = np.bincount(key, minlength=4)
        per_core.append((order, counts))
    gmax = max(int(c.max()) for _, c in per_core)
    g_sup = -(-gmax // SUP)
    e_pad = 4 * g_sup * SUP
    nsup = e_pad // SUP
    sup_parity = []
    for g in range(4):
        sup_parity += [(g >> 1, g & 1)] * g_sup

    in_maps = []
    inv_pos_all = []
    for i in range(ncores):
        sl = slice(i * ec, (i + 1) * ec)
        order, counts = per_core[i]
        s_c, r_c, l_c = senders[sl], receivers[sl], lengths[sl]
        s_p = np.zeros(e_pad, np.int64)
        r_p = np.zeros(e_pad, np.int64)
        l_p = np.zeros((e_pad, NR), np.float32)
        inv_pos = np.empty(ec, np.int64)
        start = 0
        for g in range(4):
            cnt = int(counts[g])
            idxs = order[start : start + cnt]
            blk = g * g_sup * SUP
            s_p[blk : blk + cnt] = s_c[idxs]
            r_p[blk : blk + cnt] = r_c[idxs]
            l_p[blk : blk + cnt] = l_c[idxs]
            if cnt < g_sup * SUP:
                s_p[blk + cnt : blk + g_sup * SUP] = g >> 1
                r_p[blk + cnt : blk + g_sup * SUP] = g & 1
            inv_pos[idxs] = blk + np.arange(cnt)
            start += cnt
        inv_pos_all.append(inv_pos)

        idx_s = (s_p >> 1).astype(np.int16)
        idx_r = (r_p >> 1).astype(np.int16)

        # idx tile [128, nsup*64]: per super: 32 cols senders, 32 receivers;
        # stream k's idxs wrapped [16, 32] at partitions 32k..32k+15
        def lay(a):
            b = a.reshape(nsup, 4, 32, 16)          # [sup, stream, col, lane]
            t = np.zeros((8, 16, nsup, 32), np.int16)
            t[0::2] = b.transpose(1, 3, 0, 2)        # even 16-blocks
            return t.reshape(P, nsup, 32)
        full = np.stack([lay(idx_s), lay(idx_r)], axis=2)  # [128, nsup, 2, 32]
        idx16 = np.ascontiguousarray(full.reshape(P, nsup * 64))

        l4 = l_p.reshape(nsup, 4, T, NR).transpose(0, 1, 3, 2)  # [sup,k,32,T]
        len4 = np.ascontiguousarray(l4.reshape(nsup, P, T))

        m = {"idx_all": idx16, "len4": len4}
        m.update(wcom)
        in_maps.append(m)

    b2_zero = (not np.any(wcom["b2i"])) and (not np.any(wcom["b2e"]))
    key = (e_pad, tuple(sup_parity), b2_zero)
    if key not in _NC_CACHE:
        _NC_CACHE[key] = _build(e_pad, sup_parity, b2_zero=b2_zero)
    nc = _NC_CACHE[key]

    for m in in_maps:
        m["w1a4i"] = w1a4
        m["wx4i"] = wx4i
        m["w1a4e"] = w1a4e
        m["wx4e"] = wx4e

    res = run_bass_kernel_spmd(nc, in_maps, core_ids=list(range(ncores)),
                               trace=False)
    kernel.last_result = res

    out_inv = np.empty((E, F), np.float32)
    out_ev = np.empty((E, NL), np.float32)
    for i in range(ncores):
        yi = np.asarray(res.results[i]["yinvT"])
        ye = np.asarray(res.results[i]["yevT"])
        pos = inv_pos_all[i]
        out_inv[i * ec : (i + 1) * ec] = yi.T[pos]
        out_ev[i * ec : (i + 1) * ec] = ye.T[pos]
    return out_inv, out_ev


# revision 9
# speedup vs baseline: 1.0152x; 1.0019x over previous
"""Trainium2 Bass kernel for nn_EuclideanAttentionBlock (edge gather + MLPs).

Self-contained: kernel(**inputs) -> (filter_w_inv [E,128], filter_w_ev [E,3]).

Design (per core, SPMD over 8 cores, edges sharded):
- ev_features quantized to u16 and kept SBUF-resident as a feature-major
  table [128, 25000, 2] (node pairs along d); gathered with the GPSIMD
  ap_gather ucode (no SWDGE descriptors), producing FEATURE-MAJOR data
  directly -> no PE transposes at all.
- Host sorts edges by (sender&1, receiver&1) so each super-tile uses a
  fixed parity -> gather's pair-select is a free strided view.
- sq = (ev_s - ev_r)^2 on DVE; per-degree segment sums are folded into the
  first-layer weights (Wx = segmask @ W1[32:35] * step^2).
- MLPs as f32r matmuls: h^T = W1a^T @ lengths^T + Wx^T @ sq^T (PSUM accum),
  silu+bias on ACT, out^T = W2^T @ h^T, bias on DVE. Outputs written
  transposed and host-transposed back.
"""
import numpy as np

P = 128
EV = 15
NL = 3
NR = 32
H = 128
F = 128
T = 512          # edges per stream-tile
SUP = 2048       # edges per super-tile (4 streams)
NPAIR = 25000    # node pairs (50000 nodes)
MAX_L = 3

_SEGMASK = np.zeros((EV, NL), np.float32)
_s0 = 0
for _l in range(1, MAX_L + 1):
    _d = 2 * _l + 1
    _SEGMASK[_s0 : _s0 + _d, _l - 1] = 1.0
    _s0 += _d

_NC_CACHE = {}


def _build(e_pad, sup_parity, b2_zero=False):
    """Per-core Bass program. sup_parity: list of (ps, pr) per super."""
    from contextlib import ExitStack
    import concourse.bass as bass
    import concourse.bacc as bacc
    import concourse.tile as tile
    from concourse import mybir

    f32 = mybir.dt.float32
    f32r = mybir.dt.float32r
    u16 = mybir.dt.uint16
    i16 = mybir.dt.int16
    nsup = e_pad // SUP

    nc = bacc.Bacc("TRN2", target_bir_lowering=False, debug=False)
    tab = nc.dram_tensor("tab", [P, NPAIR * 2], u16, kind="ExternalInput")
    idx_all = nc.dram_tensor("idx_all", [P, nsup * 64], i16, kind="ExternalInput")
    len4 = nc.dram_tensor("len4", [nsup, P, T], f32r, kind="ExternalInput")
    w1a4i = nc.dram_tensor("w1a4i", [P, H], f32r, kind="ExternalInput")
    wx4i = nc.dram_tensor("wx4i", [P, H], f32r, kind="ExternalInput")
    w1a4e = nc.dram_tensor("w1a4e", [P, H], f32r, kind="ExternalInput")
    wx4e = nc.dram_tensor("wx4e", [P, H], f32r, kind="ExternalInput")
    w2i = nc.dram_tensor("w2i", [H, F], f32r, kind="ExternalInput")
    w2e = nc.dram_tensor("w2e", [H, NL], f32r, kind="ExternalInput")
    b1i = nc.dram_tensor("b1i", [H, 1], f32, kind="ExternalInput")
    b1e = nc.dram_tensor("b1e", [H, 1], f32, kind="ExternalInput")
    b2i = nc.dram_tensor("b2i", [F, 1], f32, kind="ExternalInput")
    b2e = nc.dram_tensor("b2e", [NL, 1], f32, kind="ExternalInput")
    yinvT = nc.dram_tensor("yinvT", [F, e_pad], f32, kind="ExternalOutput")
    yevT = nc.dram_tensor("yevT", [NL, e_pad], f32, kind="ExternalOutput")

    with tile.TileContext(nc) as tc, ExitStack() as ctx:
        cpool = ctx.enter_context(tc.tile_pool(name="const", bufs=1))
        lpool = ctx.enter_context(tc.tile_pool(name="len", bufs=4))
        gpool = ctx.enter_context(tc.tile_pool(name="g", bufs=3))
        dpool = ctx.enter_context(tc.tile_pool(name="d", bufs=3))
        hpool = ctx.enter_context(tc.tile_pool(name="h", bufs=3))
        ypool = ctx.enter_context(tc.tile_pool(name="y", bufs=3))
        epool = ctx.enter_context(tc.tile_pool(name="yev", bufs=3))
        ps_m = ctx.enter_context(tc.tile_pool(name="ps", bufs=4, space="PSUM"))

        tab_t = cpool.tile([P, NPAIR * 2], u16)
        nc.sync.dma_start(out=tab_t[:], in_=tab[:, :])
        idx_t = cpool.tile([P, nsup * 64], i16)
        nc.sync.dma_start(out=idx_t[:], in_=idx_all[:, :])
        w1ai_t = cpool.tile([P, H], f32r)
        wxi_t = cpool.tile([P, H], f32r)
        w1ae_t = cpool.tile([P, H], f32r)
        wxe_t = cpool.tile([P, H], f32r)
        w2i_t = cpool.tile([H, F], f32r)
        w2e_t = cpool.tile([H, NL], f32r)
        b1i_t = cpool.tile([H, 1], f32)
        b1e_t = cpool.tile([H, 1], f32)
        b2i_t = cpool.tile([F, 1], f32)
        b2e_t = cpool.tile([NL, 1], f32)
        for t_, d_ in ((w1ai_t, w1a4i), (wxi_t, wx4i), (w1ae_t, w1a4e),
                       (wxe_t, wx4e), (w2i_t, w2i), (w2e_t, w2e),
                       (b1i_t, b1i), (b1e_t, b1e), (b2i_t, b2i), (b2e_t, b2e)):
            nc.sync.dma_start(out=t_[:], in_=d_[:, :])

        tab_v = tab_t[:].rearrange("p (a b) -> p a b", b=2)

        for s in range(nsup):
            ps, pr = sup_parity[s]
            len_t = lpool.tile([P, T], f32r, tag="len")
            nc.sync.dma_start(out=len_t[:], in_=len4[s])

            g_t = gpool.tile([P, T * 4], u16, tag="gs")
            nc.gpsimd.ap_gather(
                out_ap=g_t[:].rearrange("p (a b) -> p a b", b=2),
                in_ap=tab_v,
                idxs_ap=idx_t[:, s * 64 : (s + 1) * 64],
                channels=P, num_elems=NPAIR, d=2, num_idxs=2 * T,
            )
            g_v = g_t[:].rearrange("p (a b) -> p a b", b=2)
            gs_v = g_v[:, 0:T, ps]
            gr_v = g_v[:, T : 2 * T, pr]
            df_t = dpool.tile([P, T], f32, tag="df")
            nc.vector.tensor_tensor(out=df_t[:], in0=gs_v, in1=gr_v,
                                    op=mybir.AluOpType.subtract)
            sq_t = dpool.tile([P, T], f32r, tag="sq")
            nc.vector.tensor_tensor(out=sq_t[:], in0=df_t[:], in1=df_t[:],
                                    op=mybir.AluOpType.mult)

            yev_sb = epool.tile([NL, SUP], f32, tag="yev")
            for p2 in range(2):
                ks = (2 * p2, 2 * p2 + 1)
                e0 = s * SUP + ks[0] * T
                mlps = (("i", w1ai_t, wxi_t, w2i_t, b1i_t, b2i_t, F),
                        ("e", w1ae_t, wxe_t, w2e_t, b1e_t, b2e_t, NL))
                h_sbs = {}
                for tag, w1a_t, wx_t, w2_t, b1_t, b2_t, mw in mlps:
                    h_ps = ps_m.tile([H, 2 * T], f32, tag="mm", space="PSUM")
                    for j, k in enumerate(ks):
                        o = 32 * k
                        sl = slice(j * T, (j + 1) * T)
                        nc.tensor.matmul(h_ps[:, sl], lhsT=w1a_t[o : o + NR, :],
                                         rhs=len_t[o : o + NR, :],
                                         start=True, stop=False,
                                         tile_position=(o, 0))
                        nc.tensor.matmul(h_ps[:, sl], lhsT=wx_t[o : o + EV, :],
                                         rhs=sq_t[o : o + EV, :],
                                         start=False, stop=True,
                                         tile_position=(o, 0))
                    h_sb = hpool.tile([H, 2 * T], f32r, tag="h" + tag)
                    nc.scalar.activation(out=h_sb[:], in_=h_ps[:],
                                         func=mybir.ActivationFunctionType.Silu,
                                         bias=b1_t[:, :1], scale=1.0)
                    h_sbs[tag] = h_sb
                for tag, w1a_t, wx_t, w2_t, b1_t, b2_t, mw in mlps:
                    y_ps = ps_m.tile([mw, 2 * T], f32, tag="mm", space="PSUM")
                    for j in range(2):
                        sl = slice(j * T, (j + 1) * T)
                        nc.tensor.matmul(y_ps[:, sl], lhsT=w2_t[:],
                                         rhs=h_sbs[tag][:, sl],
                                         start=True, stop=True)
                    if tag == "i":
                        y_sb = ypool.tile([F, 2 * T], f32, tag="y")
                        if b2_zero:
                            nc.vector.tensor_copy(out=y_sb[:], in_=y_ps[:])
                        else:
                            nc.vector.tensor_scalar_add(out=y_sb[:], in0=y_ps[:],
                                                        scalar1=b2_t[:, :1])
                        nc.sync.dma_start(out=yinvT[:, e0 : e0 + 2 * T],
                                          in_=y_sb[:])
                    else:
                        sl2 = slice(ks[0] * T, (ks[1] + 1) * T)
                        if b2_zero:
                            nc.vector.tensor_copy(out=yev_sb[:, sl2], in_=y_ps[:])
                        else:
                            nc.vector.tensor_scalar_add(out=yev_sb[:, sl2],
                                                        in0=y_ps[:],
                                                        scalar1=b2_t[:, :1])
            nc.sync.dma_start(out=yevT[:, s * SUP : (s + 1) * SUP], in_=yev_sb[:])
    nc.compile()
    return nc


def kernel(**inputs):
    from concourse.bass_utils import run_bass_kernel_spmd

    ev = np.asarray(inputs["ev_features"], np.float32)
    senders = np.asarray(inputs["senders"]).astype(np.int64)
    receivers = np.asarray(inputs["receivers"]).astype(np.int64)
    lengths = np.asarray(inputs["lengths"], np.float32)
    E = senders.shape[0]
    import os
    ncores = int(os.environ.get("K_NCORES", "8"))
    assert E % ncores == 0
    ec = E // ncores

    # quantize ev to u16 (offset cancels in the diff; step^2 folds into Wx)
    lo = float(ev.min())
    hi = float(ev.max())
    step = (hi - lo) / 65535.0 if hi > lo else 1.0
    evq = np.clip(np.rint((ev - lo) / step), 0, 65535).astype(np.uint16)
    evqT = np.zeros((16, 50000), np.uint16)
    evqT[:EV] = evq.T
    tab128 = np.ascontiguousarray(np.tile(evqT, (8, 1)))  # [128, 50000]

    W1i = np.asarray(inputs["W1_inv"], np.float32)
    W1e = np.asarray(inputs["W1_ev"], np.float32)
    s2 = np.float32(step * step)
    wxi = (_SEGMASK @ W1i[NR:]) * s2
    wxe = (_SEGMASK @ W1e[NR:]) * s2
    w1a4 = np.zeros((P, H), np.float32)
    wx4i = np.zeros((P, H), np.float32)
    wx4e = np.zeros((P, H), np.float32)
    for k in range(4):
        w1a4[32 * k : 32 * k + NR] = W1i[:NR]
        wx4i[32 * k : 32 * k + EV] = wxi
        wx4e[32 * k : 32 * k + EV] = wxe
    w1a4e = np.zeros((P, H), np.float32)
    for k in range(4):
        w1a4e[32 * k : 32 * k + NR] = W1e[:NR]

    wcom = {
        "tab": tab128,
        "w2i": np.ascontiguousarray(np.asarray(inputs["W2_inv"], np.float32)),
        "w2e": np.ascontiguousarray(np.asarray(inputs["W2_ev"], np.float32)),
        "b1i": np.asarray(inputs["b1_inv"], np.float32).reshape(H, 1),
        "b1e": np.asarray(inputs["b1_ev"], np.float32).reshape(H, 1),
        "b2i": np.asarray(inputs["b2_inv"], np.float32).reshape(F, 1),
        "b2e": np.asarray(inputs["b2_ev"], np.float32).reshape(NL, 1),
    }

    # group per core by (s&1, r&1); common padded group size across cores
    per_core = []
    for i in range(ncores):
        sl = slice(i * ec, (i + 1) * ec)
        key = ((senders[sl] & 1) * 2 + (receivers[sl] & 1)).astype(np.int64)
        order = np.argsort(key, kind="stable")
        counts = np.bincount(key, minlength=4)
        per_core.append((order, counts))
    gmax = max(int(c.max()) for _, c in per_core)
    g_sup = -(-gmax // SUP)
    e_pad = 4 * g_sup * SUP
    nsup = e_pad // SUP
    sup_parity = []
    for g in range(4):
        sup_parity += [(g >> 1, g & 1)] * g_sup

    in_maps = []
    inv_pos_all = []
    for i in range(ncores):
        sl = slice(i * ec, (i + 1) * ec)
        order, counts = per_core[i]
        s_c, r_c, l_c = senders[sl], receivers[sl], lengths[sl]
        s_p = np.zeros(e_pad, np.int64)
        r_p = np.zeros(e_pad, np.int64)
        l_p = np.zeros((e_pad, NR), np.float32)
        inv_pos = np.empty(ec, np.int64)
        start = 0
        for g in range(4):
            cnt = int(counts[g])
            idxs = order[start : start + cnt]
            blk = g * g_sup * SUP
            s_p[blk : blk + cnt] = s_c[idxs]
            r_p[blk : blk + cnt] = r_c[idxs]
            l_p[blk : blk + cnt] = l_c[idxs]
            if cnt < g_sup * SUP:
                s_p[blk + cnt : blk + g_sup * SUP] = g >> 1
                r_p[blk + cnt : blk + g_sup * SUP] = g & 1
            inv_pos[idxs] = blk + np.arange(cnt)
            start += cnt
        inv_pos_all.append(inv_pos)

        idx_s = (s_p >> 1).astype(np.int16)
        idx_r = (r_p >> 1).astype(np.int16)

        # idx tile [128, nsup*64]: per super ONE 1024-idx stream per group:
        # [senders512 | receivers512] wrapped [16, 64] at even 16-blocks
        comb = np.concatenate([idx_s.reshape(nsup, 4, T),
                               idx_r.reshape(nsup, 4, T)], axis=2)  # [sup,k,1024]
        b = comb.reshape(nsup, 4, 64, 16)           # [sup, stream, col, lane]
        t = np.zeros((8, 16, nsup, 64), np.int16)
        t[0::2] = b.transpose(1, 3, 0, 2)
        idx16 = np.ascontiguousarray(t.reshape(P, nsup * 64))

        l4 = l_p.reshape(nsup, 4, T, NR).transpose(0, 1, 3, 2)  # [sup,k,32,T]
        len4 = np.ascontiguousarray(l4.reshape(nsup, P, T))

        m = {"idx_all": idx16, "len4": len4}
        m.update(wcom)
        in_maps.append(m)

    b2_zero = (not np.any(wcom["b2i"])) and (not np.any(wcom["b2e"]))
    key = (e_pad, tuple(sup_parity), b2_zero)
    if key not in _NC_CACHE:
        _NC_CACHE[key] = _build(e_pad, sup_parity, b2_zero=b2_zero)
    nc = _NC_CACHE[key]

    for m in in_maps:
        m["w1a4i"] = w1a4
        m["wx4i"] = wx4i
        m["w1a4e"] = w1a4e
        m["wx4e"] = wx4e

    res = run_bass_kernel_spmd(nc, in_maps, core_ids=list(range(ncores)),
                               trace=False)
    kernel.last_result = res

    out_inv = np.empty((E, F), np.float32)
    out_ev = np.empty((E, NL), np.float32)
    for i in range(ncores):
        yi = np.asarray(res.results[i]["yinvT"])
        ye = np.asarray(res.results[i]["yevT"])
        pos = inv_pos_all[i]
        out_inv[i * ec : (i + 1) * ec] = yi.T[pos]
        out_ev[i * ec : (i + 1) * ec] = ye.T[pos]
    return out_inv, out_ev
